# revision 77
# baseline (speedup 1.0000x reference)
# Trainium2 Bass kernel for nn_EpisodeMultiheadAttentionBlock.
# B=8, S=1024, E=1024, H=8 heads, HD=128. Data-parallel over batch: core b
# computes batch element b. Self-contained: only needs /opt/trn_rl_repo on path.
#
# v5 (~144us cost-model, vs 197us baseline). Key design points:
#  - RoPE pair-shuffle via SBUF->SBUF DMA (sign folded into the sin table):
#    no pmat matmuls on PE, no psum->sbuf shuffle copies on Act.
#  - The cost model serializes ALL DMA transfers into one FIFO pipe, so
#    transfer ORDER is managed explicitly: minimal early loads (xt8/Wv/Wq/
#    cos+sin tiles 0-1/Wk), rope tables streamed per-tile inside the P3
#    loop, the 9MB of late weights emitted after the last rope swap, and
#    Wyg last on SP (its buffer frees only when attention ends).
#  - P2(v)/P3(qk+rope)/P4(attention) emission is FUSED: scores/exp/av for
#    head h are emitted as soon as tile h is rotated, so attention's
#    DVE/Act/Pool ops sit early in those in-order queues; PE parking on
#    not-yet-ready scores is absorbed by P3's PE slack.
#  - bv/bo folded exactly into bo' = bo + bv@Wo on host (softmax rows sum
#    to 1, so the v-bias passes through attention additively).
#  - bq/bk/bxz/bo handled by rank-1 matmuls / Act bias only when nonzero
#    (the common setup has all-zero biases -> "zb" fast variant).
#  - z-gate x-side: 2 fp8 DoubleRow chains (hi + activation-residual);
#    1 chain fails the 2e-2 gate, Z3 restores the 3-chain variant.
#  - PSUM (8 banks): proj "a" 2 + scores "sc" 4 + den/ctx "dc" 2 during
#    the stream; P5/P6 alternate chains across a/dc for a 4-deep ladder;
#    P7 z/h reuse "sc".
#  - Elementwise spread: Act gets q-convert halves, v-dequant and all
#    activations (exp is its 43us floor); DVE gets k-convert halves, rope
#    muls/adds, reciprocal+ctx-mul, relu halves, gate combines; Pool gets
#    causal affine_select and P7's (h-x). Masked-block zeros are written
#    once per expT buffer, not once per head.
#  - P7 combine+store runs per 512-half (256 on the last tile) to shorten
#    the end-of-kernel serial chain; xf loads ride the gpsimd SWDGE queue.
import sys
import numpy as np

sys.path.insert(0, "/opt/trn_rl_repo")

import ml_dtypes  # noqa: E402
import concourse.bass as bass  # noqa: E402
import concourse.mybir as mybir  # noqa: E402
import concourse.tile as tile  # noqa: E402
from concourse import bacc  # noqa: E402
from concourse import bass_utils  # noqa: E402

B, S, E, H = 8, 1024, 1024, 8
HD = E // H  # 128
NT = E // 128  # 8 e-tiles / s-tiles
NP = NT // 2  # 4 DoubleRow k-tile pairs
NC = 8  # cores
BF16 = mybir.dt.bfloat16
F32 = mybir.dt.float32
FP8 = mybir.dt.float8e4
AF = mybir.ActivationFunctionType
DR = mybir.MatmulPerfMode.DoubleRow
ALU = mybir.AluOpType
NPBF16 = ml_dtypes.bfloat16
NPFP8 = ml_dtypes.float8_e4m3

WS = 32.0  # weight pre-scale for fp8 weights
YS = 8.0   # y stored as 8*y in fp8
CS = 16.0  # ctx stored as 16*ctx in fp8
Z3 = False  # 3-chain z x-side (precision fallback)

_COMPILED = {}


def _build(share_qk: bool, zb: bool):
    nc = bacc.Bacc("TRN2", target_bir_lowering=False, debug=False, num_devices=NC)

    # ---- DRAM tensors -------------------------------------------------------
    xb_d = nc.dram_tensor("xb", [S, E], F32, kind="ExternalInput")
    xt8_d = nc.dram_tensor("xt8", [128, NT * S], FP8, kind="ExternalInput")
    xl8_d = nc.dram_tensor("xl8", [128, NT * S], FP8, kind="ExternalInput")
    wnames = ["Wq", "Wk", "Wv", "Wo", "Wxr", "Wyr", "Wyz", "Wxg", "Wyg",
              "Wxzh", "Wxzh2"]
    if Z3:
        wnames.append("WxzR")
    w8_d = {nm: nc.dram_tensor(nm, [128, NT * E], FP8, kind="ExternalInput")
            for nm in wnames}
    if not zb:
        bo8_d = nc.dram_tensor("bo8", [128, NT], F32, kind="ExternalInput")
    ones8_d = nc.dram_tensor("ones8", [128, 256], FP8, kind="ExternalInput")
    cosq_d = nc.dram_tensor("cosq", [128, NT * S], BF16, kind="ExternalInput")
    sinq_d = nc.dram_tensor("sinq", [128, NT * S], BF16, kind="ExternalInput")
    if not share_qk:
        cosk_d = nc.dram_tensor("cosk", [128, NT * S], BF16, kind="ExternalInput")
        sink_d = nc.dram_tensor("sink", [128, NT * S], BF16, kind="ExternalInput")
    if not zb:
        bqk_row_d = nc.dram_tensor("bqk_row", [1, 2 * E], BF16, kind="ExternalInput")
        bxz_row_d = nc.dram_tensor("bxz_row", [1, E], BF16, kind="ExternalInput")
    out_d = nc.dram_tensor("out", [S, E], F32, kind="ExternalOutput")

    SCALE = 1.0 / float(np.sqrt(HD))

    def r3(ap):
        return ap.rearrange("p (t s) -> p t s", t=NT)

    with tile.TileContext(nc) as tc:
      from contextlib import ExitStack

      with ExitStack() as top:
        res = top.enter_context(tc.tile_pool(name="res", bufs=1))
        consts = top.enter_context(tc.tile_pool(name="consts", bufs=1))
        wp8 = top.enter_context(tc.tile_pool(name="wp8", bufs=8))
        psum = top.enter_context(tc.tile_pool(name="psum", bufs=1, space="PSUM"))

        def load_w8(nm):
            t = wp8.tile([128, NT, E], FP8, tag="W8", name=f"w_{nm}")
            nc.sync.dma_start(out=t, in_=w8_d[nm].ap().rearrange("p (t e) -> p t e", t=NT))
            return t

        # ------- loads in prefetch order: what P2 needs first ---------------
        ones8 = consts.tile([128, 2, 128], FP8, tag="ones8")  # value 1/CS
        bo8_sb = None
        if not zb:
            bqk_row = consts.tile([1, 2 * E], BF16, tag="bqk_row")
            nc.sync.dma_start(out=bqk_row, in_=bqk_row_d.ap())
            bxz_row = consts.tile([1, E], BF16, tag="bxz_row")
            nc.sync.dma_start(out=bxz_row, in_=bxz_row_d.ap())
            ones1_b = consts.tile([1, 1024], BF16, tag="ones1_b")
            nc.vector.memset(ones1_b, 1.0)

        xT8 = res.tile([128, NT, S], FP8, tag="xT8")
        xt8_r = r3(xt8_d.ap())
        nc.sync.dma_start(out=xT8[:, 0:4, :], in_=xt8_r[:, 0:4, :])
        wv_sb = wp8.tile([128, NT, E], FP8, tag="W8", name="w_Wv")
        wv_r = w8_d["Wv"].ap().rearrange("p (t e) -> p t e", t=NT)
        nc.sync.dma_start(out=wv_sb[:, :, 0:512], in_=wv_r[:, :, 0:512])
        nc.sync.dma_start(out=xT8[:, 4:NT, :], in_=xt8_r[:, 4:NT, :])
        nc.sync.dma_start(out=wv_sb[:, :, 512:E], in_=wv_r[:, :, 512:E])
        wq_sb = load_w8("Wq")

        # mid tiles live through P4 only
        mid_ctx = tc.tile_pool(name="mid", bufs=1)
        mid = mid_ctx.__enter__()
        vsb8 = mid.tile([128, NT, E], FP8, tag="vsb8")   # v in fp8  [s, e]
        qr = mid.tile([128, NT, S], BF16, tag="qr")      # rope(q)^T
        kr = mid.tile([128, NT, S], BF16, tag="kr")      # rope(k)^T
        # rope tables live through P3 only; tiles 0-1 land before wk so the
        # first rotates (and thus head 0 of P4) start as early as possible
        tab_ctx = tc.tile_pool(name="tab", bufs=1)
        tab = tab_ctx.__enter__()
        cosq = tab.tile([128, NT, S], BF16, tag="cosq")
        sinq = tab.tile([128, NT, S], BF16, tag="sinq")
        cosq_r, sinq_r = r3(cosq_d.ap()), r3(sinq_d.ap())
        nc.sync.dma_start(out=cosq[:, 0:2, :], in_=cosq_r[:, 0:2, :])
        nc.sync.dma_start(out=sinq[:, 0:2, :], in_=sinq_r[:, 0:2, :])
        wk_sb = load_w8("Wk")
        # small consts off the latency-critical pipe front (first use ~35us)
        nc.sync.dma_start(out=ones8, in_=ones8_d.ap().rearrange("p (a b) -> p a b", a=2))
        if not zb:
            bo8_sb = consts.tile([128, NT], F32, tag="bo8")
            nc.sync.dma_start(out=bo8_sb, in_=bo8_d.ap())

        def load_tab_chunk(a, b):
            nc.sync.dma_start(out=cosq[:, a:b, :], in_=cosq_r[:, a:b, :])
            nc.sync.dma_start(out=sinq[:, a:b, :], in_=sinq_r[:, a:b, :])

        # Late tiles: allocated now, DMAs emitted mid-P3 behind a marker so
        # the serial DMA pipe stays clear for rope tables/swaps early on.
        wo_sb = wp8.tile([128, NT, E], FP8, tag="W8", name="w_Wo")
        wxr_sb = wp8.tile([128, NT, E], FP8, tag="W8", name="w_Wxr")
        wyr_sb = wp8.tile([128, NT, E], FP8, tag="W8", name="w_Wyr")
        wxzh_sb = wp8.tile([128, NT, E], FP8, tag="W8", name="w_Wxzh")
        wxzh2_sb = wp8.tile([128, NT, E], FP8, tag="W8", name="w_Wxzh2")
        wxzr_sb = wp8.tile([128, NT, E], FP8, tag="W8", name="w_WxzR") if Z3 else None
        wyz_sb = wp8.tile([128, NT, E], FP8, tag="W8", name="w_Wyz")
        wxg_sb = wp8.tile([128, NT, E], FP8, tag="W8", name="w_Wxg")
        wyg_sb = wp8.tile([128, NT, E], FP8, tag="W8", name="w_Wyg")
        xl8 = res.tile([128, NT, S], FP8, tag="xl8")

        def emit_late_loads():
            # Emitted after the P3 loop: SP-queue FIFO order keeps these 9MB
            # behind the latency-critical rope swap DMAs on the serial pipe.
            pairs = [(wo_sb, "Wo"), (wxr_sb, "Wxr"), (wyr_sb, "Wyr"),
                     (xl8, None), (wxzh_sb, "Wxzh"), (wxzh2_sb, "Wxzh2")]
            if Z3:
                pairs.append((wxzr_sb, "WxzR"))
            pairs += [(wyz_sb, "Wyz"), (wxg_sb, "Wxg"), (wyg_sb, "Wyg")]
            for t, nm in pairs:
                if nm is None:
                    nc.sync.dma_start(out=t, in_=r3(xl8_d.ap()))
                else:
                    nc.sync.dma_start(
                        out=t, in_=w8_d[nm].ap().rearrange("p (t e) -> p t e", t=NT))

        # ===== P2: v = x @ Wv  (seq-major, fp8 out; dequant split Act/DVE) ==
        for st in range(NT):
            ss = slice(st * 128, (st + 1) * 128)
            ps = psum.tile([128, 1024], F32, tag="sc", bufs=2, name="ps_v")
            for c in range(2):
                sl = slice(c * 512, (c + 1) * 512)
                for kp in range(NP):
                    nc.tensor.matmul(
                        ps[:, sl], lhsT=xT8[:, 2 * kp:2 * kp + 2, ss],
                        rhs=wv_sb[:, 2 * kp:2 * kp + 2, sl],
                        start=(kp == 0), stop=(kp == NP - 1), perf_mode=DR)
            nc.scalar.activation(vsb8[:, st, :], ps, AF.Copy, scale=1.0 / WS)

        # ====== P3+P4 fused: q/k proj + RoPE, with per-head attention ======
        # Emitting scores/exp/av for head h as soon as tile h is rotated puts
        # the attention's DVE/Pool/Act ops early in those engines' in-order
        # queues; PE parking on not-yet-ready scores is absorbed by P3's PE
        # slack (rope is DVE/DMA-paced).
        p3_ctx = tc.tile_pool(name="p3", bufs=1)
        p3 = p3_ctx.__enter__()
        p4_ctx = tc.tile_pool(name="p4", bufs=1)
        p4 = p4_ctx.__enter__()
        ctx8 = res.tile([128, NT, S], FP8, tag="ctx8")  # 16*ctx in fp8
        EXPB = 3  # expT ring size; masked-block zeros written once/buffer

        def emit_scores(h):
            expT = p4.tile([128, NT, S], FP8, tag="expT", bufs=EXPB)
            for jt in range(NT):
                i0 = jt * 128
                ps = psum.tile([128, 1024], F32, tag="sc", bufs=2, name="ps_sc")
                if i0 < 512:
                    nc.tensor.matmul(
                        ps[:, i0:512],
                        lhsT=kr[:, h, i0:i0 + 128],
                        rhs=qr[:, h, i0:512], start=True, stop=True)
                    nc.tensor.matmul(
                        ps[:, 512:1024],
                        lhsT=kr[:, h, i0:i0 + 128],
                        rhs=qr[:, h, 512:1024], start=True, stop=True)
                else:
                    nc.tensor.matmul(
                        ps[:, i0:1024],
                        lhsT=kr[:, h, i0:i0 + 128],
                        rhs=qr[:, h, i0:1024], start=True, stop=True)
                nc.scalar.activation(expT[:, jt, i0:1024], ps[:, i0:1024],
                                     AF.Exp, scale=SCALE)
                nc.gpsimd.affine_select(
                    out=expT[:, jt, i0:i0 + 128], in_=expT[:, jt, i0:i0 + 128],
                    pattern=[[1, 128]], compare_op=ALU.is_ge,
                    fill=0.0, base=0, channel_multiplier=-1)
                if jt % 2 == 1 and h < EXPB:
                    nc.vector.memset(expT[:, jt, i0 - 128:i0], 0.0)
            return expT

        def emit_av(h, expT):
            for c in range(2):
                cs, ce = c * 512, (c + 1) * 512
                jps = [jp for jp in range(NP) if jp * 256 < ce]
                dps = psum.tile([128, 512], F32, tag="dc", bufs=2, name="ps_den")
                for n, jp in enumerate(jps):
                    a = max(jp * 256, cs)
                    nc.tensor.matmul(
                        dps[:, a - cs:512], lhsT=ones8,
                        rhs=expT[:, 2 * jp:2 * jp + 2, a:ce],
                        start=(n == 0), stop=(n == len(jps) - 1), perf_mode=DR)
                rf = p4.tile([128, 512], F32, tag="rf", bufs=1)
                nc.vector.reciprocal_approx_fast(out=rf, in_=dps)
                cps = psum.tile([128, 512], F32, tag="dc", bufs=2, name="ps_ctx")
                for n, jp in enumerate(jps):
                    a = max(jp * 256, cs)
                    nc.tensor.matmul(
                        cps[:, a - cs:512],
                        lhsT=vsb8[:, 2 * jp:2 * jp + 2, h * 128:(h + 1) * 128],
                        rhs=expT[:, 2 * jp:2 * jp + 2, a:ce],
                        start=(n == 0), stop=(n == len(jps) - 1), perf_mode=DR)
                nc.vector.tensor_mul(ctx8[:, h, cs:ce], cps, rf)

        if True:
            def proj_tile(t, w_sb, brow_off, on_act=True):
                # psum chains 512-wide; convert to bf16 qs; DMA pair-swap
                qs = p3.tile([128, S], BF16, tag="qs", bufs=3)
                for c in range(2):
                    sl = slice(c * 512, (c + 1) * 512)
                    ps = psum.tile([128, 512], F32, tag="a", bufs=2, name="ps_qk")
                    for kp in range(NP):
                        nc.tensor.matmul(
                            ps, lhsT=w_sb[:, 2 * kp:2 * kp + 2, t * 128:(t + 1) * 128],
                            rhs=xT8[:, 2 * kp:2 * kp + 2, sl],
                            start=(kp == 0), stop=(kp == NP - 1 and zb), perf_mode=DR)
                    if not zb:
                        nc.tensor.matmul(
                            ps, lhsT=bqk_row[:, brow_off + t * 128:brow_off + (t + 1) * 128],
                            rhs=ones1_b[:, sl], start=False, stop=True)
                    if c == (0 if on_act else 1):
                        nc.scalar.activation(qs[:, sl], ps, AF.Copy, scale=1.0 / WS)
                    else:
                        nc.vector.tensor_scalar(out=qs[:, sl], in0=ps,
                                                scalar1=1.0 / WS, scalar2=None,
                                                op0=ALU.mult)
                qss = p3.tile([128, S], BF16, tag="qss", bufs=3)
                qs_r = qs[:].rearrange("(h two) f -> h two f", two=2)
                qss_r = qss[:].rearrange("(h two) f -> h two f", two=2)
                nc.sync.dma_start(out=qss_r[:, 0, :], in_=qs_r[:, 1, :])
                nc.sync.dma_start(out=qss_r[:, 1, :], in_=qs_r[:, 0, :])
                return qs, qss

            def rotate(t, qs, qss, cos_t, sin_t, dst, add_on_pool=True):
                t1 = p3.tile([128, S], BF16, tag="t1", bufs=2)
                nc.vector.tensor_mul(t1, qs, cos_t[:, t, :])
                t2 = p3.tile([128, S], BF16, tag="t2", bufs=2)
                nc.vector.tensor_mul(t2, qss, sin_t[:, t, :])
                if add_on_pool:
                    nc.gpsimd.tensor_add(dst[:, t, :], t1, t2)
                else:
                    nc.vector.tensor_add(dst[:, t, :], t1, t2)

            pend_h = []

            def head_ready(r):
                pend_h.append((r, emit_scores(r)))
                if len(pend_h) > 2:
                    ph, pexp = pend_h.pop(0)
                    emit_av(ph, pexp)

            if share_qk:
                pend = []
                for t in range(NT):
                    q_t = proj_tile(t, wq_sb, 0, on_act=True)
                    k_t = proj_tile(t, wk_sb, E, on_act=False)
                    if 0 <= t < 6:
                        load_tab_chunk(t + 2, t + 3)
                    pend.append((t, q_t, k_t))
                    if len(pend) > 2:  # distance-2: swap DMA latency headroom
                        pt, pq, pk = pend.pop(0)
                        rotate(pt, pq[0], pq[1], cosq, sinq, qr, False)
                        rotate(pt, pk[0], pk[1], cosq, sinq, kr, False)
                        head_ready(pt)
                for pt, pq, pk in pend:
                    rotate(pt, pq[0], pq[1], cosq, sinq, qr, False)
                    rotate(pt, pk[0], pk[1], cosq, sinq, kr, False)
                    head_ready(pt)
                emit_late_loads()
                for ph, pexp in pend_h:
                    emit_av(ph, pexp)
            else:
                for t in range(NT):
                    q_t = proj_tile(t, wq_sb, 0)
                    if t == 0:
                        load_tab_chunk(2, 5)
                    if t == 2:
                        load_tab_chunk(5, NT)
                    rotate(t, q_t[0], q_t[1], cosq, sinq, qr, False)
                    if t == NT - 1:
                        emit_late_loads()
                cosk = tab.tile([128, NT, S], BF16, tag="cosq")
                nc.sync.dma_start(out=cosk, in_=r3(cosk_d.ap()))
                sink = tab.tile([128, NT, S], BF16, tag="sinq")
                nc.sync.dma_start(out=sink, in_=r3(sink_d.ap()))
                for t in range(NT):
                    k_t = proj_tile(t, wk_sb, E)
                    rotate(t, k_t[0], k_t[1], cosk, sink, kr)
                    head_ready(t)
                for ph, pexp in pend_h:
                    emit_av(ph, pexp)

        p4_ctx.__exit__(None, None, None)
        p3_ctx.__exit__(None, None, None)
        tab_ctx.__exit__(None, None, None)

        # =========== P5: y8 = 8*relu(ctx Wo + bo')  (feature-major) =========
        mid_ctx.__exit__(None, None, None)
        res2 = top.enter_context(tc.tile_pool(name="res2", bufs=1))
        yT8 = res2.tile([128, NT, S], FP8, tag="yT8")    # 8*y in fp8
        rx = res2.tile([128, NT, S], FP8, tag="rx")      # (r*x)^T fp8
        for t in range(NT):
            for c in range(2):
                sl = slice(c * 512, (c + 1) * 512)
                ps = psum.tile([128, 512], F32, tag=("a" if c == 0 else "dc"),
                               bufs=2, name="ps_y")
                for kp in range(NP):
                    nc.tensor.matmul(
                        ps, lhsT=wo_sb[:, 2 * kp:2 * kp + 2, t * 128:(t + 1) * 128],
                        rhs=ctx8[:, 2 * kp:2 * kp + 2, sl],
                        start=(kp == 0), stop=(kp == NP - 1), perf_mode=DR)
                if zb:
                    # bo' == 0: relu(s*ps) = s*max(ps, 0); alternate engines
                    # so the post-attention relu ladder runs two-wide
                    if c == 0:
                        nc.scalar.activation(yT8[:, t, sl], ps, AF.Relu,
                                             scale=YS / (WS * CS))
                    else:
                        nc.vector.tensor_scalar(out=yT8[:, t, sl], in0=ps,
                                                scalar1=0.0, scalar2=YS / (WS * CS),
                                                op0=ALU.max, op1=ALU.mult)
                else:
                    nc.scalar.activation(yT8[:, t, sl], ps, AF.Relu,
                                         bias=bo8_sb[:, t:t + 1],
                                         scale=YS / (WS * CS))

        # ===== P6: r = sigmoid(x Wxr + y Wyr); rx = r * xT (bf16) ============
        with tc.tile_pool(name="p6", bufs=1) as p6:
            for t in range(NT):
                rt = p6.tile([128, S], BF16, tag="rt", bufs=2)
                for c in range(2):
                    sl = slice(c * 512, (c + 1) * 512)
                    ps = psum.tile([128, 512], F32, tag=("a" if c == 0 else "dc"),
                                   bufs=2, name="ps_r")
                    for kp in range(NP):
                        nc.tensor.matmul(
                            ps, lhsT=wxr_sb[:, 2 * kp:2 * kp + 2, t * 128:(t + 1) * 128],
                            rhs=xT8[:, 2 * kp:2 * kp + 2, sl],
                            start=(kp == 0), stop=False, perf_mode=DR)
                    for kp in range(NP):
                        nc.tensor.matmul(
                            ps, lhsT=wyr_sb[:, 2 * kp:2 * kp + 2, t * 128:(t + 1) * 128],
                            rhs=yT8[:, 2 * kp:2 * kp + 2, sl],
                            start=False, stop=(kp == NP - 1), perf_mode=DR)
                    nc.scalar.activation(rt[:, sl], ps, AF.Sigmoid, scale=1.0 / WS)
                nc.vector.tensor_mul(rx[:, t, :], rt, xT8[:, t, :])

        # =========== P7: z/h + gated combine (seq-major, single pass) ========
        # xf loads + out stores ride the Act HWDGE queue to dodge SP-queue
        # head-of-line blocking behind the late weight loads.
        with tc.tile_pool(name="p7", bufs=1) as p7:
            xfs = []

            def load_xf(st):
                xf = p7.tile([128, E], F32, tag="xf", bufs=3)
                nc.gpsimd.dma_start(out=xf, in_=xb_d.ap()[st * 128:(st + 1) * 128, :])
                xfs.append(xf)

            load_xf(0)
            load_xf(1)
            for st in range(NT):
                ss = slice(st * 128, (st + 1) * 128)
                if st + 2 < NT:
                    load_xf(st + 2)
                xf = xfs[st]
                ot = p7.tile([128, E], F32, tag="ot", bufs=2)
                zps = psum.tile([128, 1024], F32, tag="sc", bufs=2, name="ps_z")
                for c in range(2):
                    sl = slice(c * 512, (c + 1) * 512)
                    chains = [(xT8, wxzh_sb), (xl8, wxzh2_sb)]
                    if Z3:
                        chains.append((xT8, wxzr_sb))
                    chains.append((yT8, wyz_sb))
                    for n, (lt, wt) in enumerate(chains):
                        for kp in range(NP):
                            nc.tensor.matmul(
                                zps[:, sl], lhsT=lt[:, 2 * kp:2 * kp + 2, ss],
                                rhs=wt[:, 2 * kp:2 * kp + 2, sl],
                                start=(n == 0 and kp == 0),
                                stop=(zb and n == len(chains) - 1 and kp == NP - 1),
                                perf_mode=DR)
                    if not zb:
                        nc.tensor.matmul(zps[:, sl], lhsT=ones1_b[:, 0:128],
                                         rhs=bxz_row[:, sl], start=False, stop=True)
                zt = p7.tile([128, E], F32, tag="zt", bufs=2)
                nc.scalar.activation(zt, zps, AF.Sigmoid, scale=1.0 / WS)
                hps = psum.tile([128, 1024], F32, tag="sc", bufs=2, name="ps_h")
                for c in range(2):
                    sl = slice(c * 512, (c + 1) * 512)
                    for kp in range(NP):
                        nc.tensor.matmul(hps[:, sl], lhsT=rx[:, 2 * kp:2 * kp + 2, ss],
                                         rhs=wxg_sb[:, 2 * kp:2 * kp + 2, sl],
                                         start=(kp == 0), stop=False, perf_mode=DR)
                    for kp in range(NP):
                        nc.tensor.matmul(hps[:, sl], lhsT=yT8[:, 2 * kp:2 * kp + 2, ss],
                                         rhs=wyg_sb[:, 2 * kp:2 * kp + 2, sl],
                                         start=False, stop=(kp == NP - 1), perf_mode=DR)
                # combine + store per 512-half: the tail (tanh -> diff ->
                # gate -> add -> store) pipelines across halves, cutting the
                # end-of-kernel serial chain roughly in half
                ht = p7.tile([128, E], F32, tag="ht", bufs=2)
                dt = p7.tile([128, E], F32, tag="dt", bufs=2)
                zd = p7.tile([128, E], F32, tag="zd", bufs=2)
                # last tile's tail chain runs at 256 granularity so the
                # final store starts as early as possible
                W = 512 if st < NT - 1 else 256
                for c in range(E // W):
                    sl = slice(c * W, (c + 1) * W)
                    nc.scalar.activation(ht[:, sl], hps[:, sl], AF.Tanh,
                                         scale=1.0 / WS)
                    if st < NT - 1:
                        nc.gpsimd.tensor_sub(dt[:, sl], ht[:, sl], xf[:, sl])
                    else:
                        nc.vector.tensor_sub(dt[:, sl], ht[:, sl], xf[:, sl])
                    nc.vector.tensor_mul(zd[:, sl], zt[:, sl], dt[:, sl])
                    nc.vector.tensor_add(ot[:, sl], xf[:, sl], zd[:, sl])
                    nc.sync.dma_start(out=out_d.ap()[ss, sl], in_=ot[:, sl])

    nc.compile()
    return nc


# ---------------- host-side packing -----------------------------------------

def _pack_w(w, scale, npdt):
    return np.ascontiguousarray(
        (np.asarray(w, np.float32) * scale).astype(npdt)
        .reshape(NT, 128, E).transpose(1, 0, 2).reshape(128, NT * E))


def _pack_fm(m, npdt):
    # [E, S]-logical feature-major -> [128, NT*S]
    return np.ascontiguousarray(
        m.astype(npdt).reshape(NT, 128, S).transpose(1, 0, 2).reshape(128, NT * S))


def _pack_bias_fm(b, scale=1.0):
    return np.ascontiguousarray(
        (np.asarray(b, np.float32) * scale).reshape(NT, 128).T)


_INV = None


def _inv_pair():
    global _INV
    if _INV is None:
        inv = 1.0 / (10000.0 ** (np.arange(0, E, 2, dtype=np.float32) / np.float32(E)))
        _INV = np.repeat(inv.astype(np.float64), 2)  # pair-expanded [E]
    return _INV


_SIGN = None


def _sign_pair():
    # sin sign pattern: -1 on even features, +1 on odd (shuffle drops the
    # pmat negation, so the sin table carries it)
    global _SIGN
    if _SIGN is None:
        s = np.ones(E, np.float64)
        s[0::2] = -1.0
        _SIGN = s
    return _SIGN


def _tables(idx):
    f = _inv_pair()[:, None] * idx.astype(np.float64)[None, :]  # [E, S]
    return (_pack_fm(np.cos(f).astype(np.float32), NPBF16),
            _pack_fm((np.sin(f) * _sign_pair()[:, None]).astype(np.float32), NPBF16))


def make_in_maps(inputs, share_qk, zb):
    x = np.asarray(inputs["x"], dtype=np.float32)
    qi = np.asarray(inputs["query_index"])
    ki = np.asarray(inputs["key_index"])
    bo_eff = (np.asarray(inputs["bo"], np.float32)
              + np.asarray(inputs["bv"], np.float32)
              @ np.asarray(inputs["Wo"], np.float32))
    common = {
        "ones8": np.full((128, 256), 1.0 / CS, NPFP8),
    }
    if not zb:
        common["bo8"] = _pack_bias_fm(bo_eff, YS)
        bqk = np.concatenate([np.asarray(inputs["bq"], np.float32),
                              np.asarray(inputs["bk"], np.float32)])
        common["bqk_row"] = (bqk * WS).astype(NPBF16).reshape(1, 2 * E)
        common["bxz_row"] = (np.asarray(inputs["bxz"], np.float32) * WS
                             ).astype(NPBF16).reshape(1, E)
    for nm in ("Wq", "Wk", "Wv", "Wo", "Wxr", "Wxg"):
        common[nm] = _pack_w(inputs[nm], WS, NPFP8)
    for nm in ("Wyr", "Wyz", "Wyg"):
        common[nm] = _pack_w(inputs[nm], WS / YS, NPFP8)
    wxz = np.asarray(inputs["Wxz"], np.float32)
    whi8 = (wxz * WS).astype(NPFP8)
    common["Wxzh"] = _pack_w(whi8.astype(np.float32), 1.0, NPFP8)
    common["Wxzh2"] = _pack_w(wxz, 2.0, NPFP8)
    if Z3:
        common["WxzR"] = _pack_w(wxz * WS - whi8.astype(np.float32), 1.0, NPFP8)
    in_maps = []
    for b in range(B):
        m = dict(common)
        xb = np.ascontiguousarray(x[b])
        m["xb"] = xb
        xt = xb.T  # [E, S]
        xh8 = xt.astype(NPFP8)
        m["xt8"] = _pack_fm(xh8.astype(np.float32), NPFP8)
        m["xl8"] = _pack_fm((xt - xh8.astype(np.float32)) * 16.0, NPFP8)
        m["cosq"], m["sinq"] = _tables(qi[b])
        if not share_qk:
            m["cosk"], m["sink"] = _tables(ki[b])
        in_maps.append(m)
    return in_maps


def kernel(**inputs):
    qi = np.asarray(inputs["query_index"])
    ki = np.asarray(inputs["key_index"])
    share_qk = bool(np.array_equal(qi, ki))
    zb = not (np.any(np.asarray(inputs["bq"])) or np.any(np.asarray(inputs["bk"]))
              or np.any(np.asarray(inputs["bxz"])) or np.any(np.asarray(inputs["bo"]))
              or np.any(np.asarray(inputs["bv"])))

    key = ("k", share_qk, zb)
    if key not in _COMPILED:
        _COMPILED[key] = _build(share_qk, zb)
    nc = _COMPILED[key]

    in_maps = make_in_maps(inputs, share_qk, zb)
    global _dbg_in_maps
    _dbg_in_maps = in_maps
    res = bass_utils.run_bass_kernel_spmd(nc, in_maps, core_ids=list(range(NC)))
    out = np.stack([res.results[b]["out"] for b in range(B)]).astype(np.float32)
    return out


# revision 81
# speedup vs baseline: 1.0005x; 1.0005x over previous
# Trainium2 Bass kernel for nn_EpisodeMultiheadAttentionBlock.
# B=8, S=1024, E=1024, H=8 heads, HD=128. Data-parallel over batch: core b
# computes batch element b. Self-contained: only needs /opt/trn_rl_repo on path.
#
# v5 (~144us cost-model, vs 197us baseline). Key design points:
#  - RoPE pair-shuffle via SBUF->SBUF DMA (sign folded into the sin table):
#    no pmat matmuls on PE, no psum->sbuf shuffle copies on Act.
#  - The cost model serializes ALL DMA transfers into one FIFO pipe, so
#    transfer ORDER is managed explicitly: minimal early loads (xt8/Wv/Wq/
#    cos+sin tiles 0-1/Wk), rope tables streamed per-tile inside the P3
#    loop, the 9MB of late weights emitted after the last rope swap, and
#    Wyg last on SP (its buffer frees only when attention ends).
#  - P2(v)/P3(qk+rope)/P4(attention) emission is FUSED: scores/exp/av for
#    head h are emitted as soon as tile h is rotated, so attention's
#    DVE/Act/Pool ops sit early in those in-order queues; PE parking on
#    not-yet-ready scores is absorbed by P3's PE slack.
#  - bv/bo folded exactly into bo' = bo + bv@Wo on host (softmax rows sum
#    to 1, so the v-bias passes through attention additively).
#  - bq/bk/bxz/bo handled by rank-1 matmuls / Act bias only when nonzero
#    (the common setup has all-zero biases -> "zb" fast variant).
#  - z-gate x-side: 2 fp8 DoubleRow chains (hi + activation-residual);
#    1 chain fails the 2e-2 gate, Z3 restores the 3-chain variant.
#  - PSUM (8 banks): proj "a" 2 + scores "sc" 4 + den/ctx "dc" 2 during
#    the stream; P5/P6 alternate chains across a/dc for a 4-deep ladder;
#    P7 z/h reuse "sc".
#  - Elementwise spread: Act gets q-convert halves, v-dequant and all
#    activations (exp is its 43us floor); DVE gets k-convert halves, rope
#    muls/adds, reciprocal+ctx-mul, relu halves, gate combines; Pool gets
#    causal affine_select and P7's (h-x). Masked-block zeros are written
#    once per expT buffer, not once per head.
#  - P7 combine+store runs per 512-half (256 on the last tile) to shorten
#    the end-of-kernel serial chain; xf loads ride the gpsimd SWDGE queue.
import sys
import numpy as np

sys.path.insert(0, "/opt/trn_rl_repo")

import ml_dtypes  # noqa: E402
import concourse.bass as bass  # noqa: E402
import concourse.mybir as mybir  # noqa: E402
import concourse.tile as tile  # noqa: E402
from concourse import bacc  # noqa: E402
from concourse import bass_utils  # noqa: E402

B, S, E, H = 8, 1024, 1024, 8
HD = E // H  # 128
NT = E // 128  # 8 e-tiles / s-tiles
NP = NT // 2  # 4 DoubleRow k-tile pairs
NC = 8  # cores
BF16 = mybir.dt.bfloat16
F32 = mybir.dt.float32
FP8 = mybir.dt.float8e4
AF = mybir.ActivationFunctionType
DR = mybir.MatmulPerfMode.DoubleRow
ALU = mybir.AluOpType
NPBF16 = ml_dtypes.bfloat16
NPFP8 = ml_dtypes.float8_e4m3

WS = 32.0  # weight pre-scale for fp8 weights
YS = 8.0   # y stored as 8*y in fp8
CS = 16.0  # ctx stored as 16*ctx in fp8
Z3 = False  # 3-chain z x-side (precision fallback)

_COMPILED = {}


def _build(share_qk: bool, zb: bool):
    nc = bacc.Bacc("TRN2", target_bir_lowering=False, debug=False, num_devices=NC)

    # ---- DRAM tensors -------------------------------------------------------
    xb_d = nc.dram_tensor("xb", [S, E], F32, kind="ExternalInput")
    xt8_d = nc.dram_tensor("xt8", [128, NT * S], FP8, kind="ExternalInput")
    xl8_d = nc.dram_tensor("xl8", [128, NT * S], FP8, kind="ExternalInput")
    wnames = ["Wq", "Wk", "Wv", "Wo", "Wxr", "Wyr", "Wyz", "Wxg", "Wyg",
              "Wxzh", "Wxzh2"]
    if Z3:
        wnames.append("WxzR")
    w8_d = {nm: nc.dram_tensor(nm, [128, NT * E], FP8, kind="ExternalInput")
            for nm in wnames}
    if not zb:
        bo8_d = nc.dram_tensor("bo8", [128, NT], F32, kind="ExternalInput")
    ones8_d = nc.dram_tensor("ones8", [128, 256], FP8, kind="ExternalInput")
    cosq_d = nc.dram_tensor("cosq", [128, NT * S], BF16, kind="ExternalInput")
    sinq_d = nc.dram_tensor("sinq", [128, NT * S], BF16, kind="ExternalInput")
    if not share_qk:
        cosk_d = nc.dram_tensor("cosk", [128, NT * S], BF16, kind="ExternalInput")
        sink_d = nc.dram_tensor("sink", [128, NT * S], BF16, kind="ExternalInput")
    if not zb:
        bqk_row_d = nc.dram_tensor("bqk_row", [1, 2 * E], BF16, kind="ExternalInput")
        bxz_row_d = nc.dram_tensor("bxz_row", [1, E], BF16, kind="ExternalInput")
    out_d = nc.dram_tensor("out", [S, E], F32, kind="ExternalOutput")

    SCALE = 1.0 / float(np.sqrt(HD))

    def r3(ap):
        return ap.rearrange("p (t s) -> p t s", t=NT)

    with tile.TileContext(nc) as tc:
      from contextlib import ExitStack

      with ExitStack() as top:
        res = top.enter_context(tc.tile_pool(name="res", bufs=1))
        consts = top.enter_context(tc.tile_pool(name="consts", bufs=1))
        wp8 = top.enter_context(tc.tile_pool(name="wp8", bufs=8))
        psum = top.enter_context(tc.tile_pool(name="psum", bufs=1, space="PSUM"))

        def load_w8(nm):
            t = wp8.tile([128, NT, E], FP8, tag="W8", name=f"w_{nm}")
            nc.sync.dma_start(out=t, in_=w8_d[nm].ap().rearrange("p (t e) -> p t e", t=NT))
            return t

        # ------- loads in prefetch order: what P2 needs first ---------------
        ones8 = consts.tile([128, 2, 128], FP8, tag="ones8")  # value 1/CS
        bo8_sb = None
        if not zb:
            bqk_row = consts.tile([1, 2 * E], BF16, tag="bqk_row")
            nc.sync.dma_start(out=bqk_row, in_=bqk_row_d.ap())
            bxz_row = consts.tile([1, E], BF16, tag="bxz_row")
            nc.sync.dma_start(out=bxz_row, in_=bxz_row_d.ap())
            ones1_b = consts.tile([1, 1024], BF16, tag="ones1_b")
            nc.vector.memset(ones1_b, 1.0)

        # PE p-state warmup: dummy matmuls fill the initial DMA-wait gap so
        # the first real chains run at full clock (ramp needs ~3us busy)
        wu = consts.tile([128, 512], BF16, tag="wu")
        nc.vector.memset(wu[:, 0:512], 0.0)
        for _ in range(14):
            wps = psum.tile([128, 512], F32, tag="a", bufs=2, name="ps_wu")
            nc.tensor.matmul(wps, lhsT=wu[:, 0:128], rhs=wu, start=True, stop=True)

        xT8 = res.tile([128, NT, S], FP8, tag="xT8")
        xt8_r = r3(xt8_d.ap())
        nc.sync.dma_start(out=xT8[:, 0:4, :], in_=xt8_r[:, 0:4, :])
        wv_sb = wp8.tile([128, NT, E], FP8, tag="W8", name="w_Wv")
        wv_r = w8_d["Wv"].ap().rearrange("p (t e) -> p t e", t=NT)
        nc.sync.dma_start(out=wv_sb[:, :, 0:512], in_=wv_r[:, :, 0:512])
        nc.sync.dma_start(out=xT8[:, 4:NT, :], in_=xt8_r[:, 4:NT, :])
        nc.sync.dma_start(out=wv_sb[:, :, 512:E], in_=wv_r[:, :, 512:E])
        wq_sb = load_w8("Wq")

        # mid tiles live through P4 only
        mid_ctx = tc.tile_pool(name="mid", bufs=1)
        mid = mid_ctx.__enter__()
        vsb8 = mid.tile([128, NT, E], FP8, tag="vsb8")   # v in fp8  [s, e]
        qr = mid.tile([128, NT, S], BF16, tag="qr")      # rope(q)^T
        kr = mid.tile([128, NT, S], BF16, tag="kr")      # rope(k)^T
        # rope tables live through P3 only; tiles 0-1 land before wk so the
        # first rotates (and thus head 0 of P4) start as early as possible
        tab_ctx = tc.tile_pool(name="tab", bufs=1)
        tab = tab_ctx.__enter__()
        cosq = tab.tile([128, NT, S], BF16, tag="cosq")
        sinq = tab.tile([128, NT, S], BF16, tag="sinq")
        cosq_r, sinq_r = r3(cosq_d.ap()), r3(sinq_d.ap())
        nc.sync.dma_start(out=cosq[:, 0:2, :], in_=cosq_r[:, 0:2, :])
        nc.sync.dma_start(out=sinq[:, 0:2, :], in_=sinq_r[:, 0:2, :])
        wk_sb = load_w8("Wk")
        # small consts off the latency-critical pipe front (first use ~35us)
        nc.sync.dma_start(out=ones8, in_=ones8_d.ap().rearrange("p (a b) -> p a b", a=2))
        if not zb:
            bo8_sb = consts.tile([128, NT], F32, tag="bo8")
            nc.sync.dma_start(out=bo8_sb, in_=bo8_d.ap())

        def load_tab_chunk(a, b):
            nc.sync.dma_start(out=cosq[:, a:b, :], in_=cosq_r[:, a:b, :])
            nc.sync.dma_start(out=sinq[:, a:b, :], in_=sinq_r[:, a:b, :])

        # Late tiles: allocated now, DMAs emitted mid-P3 behind a marker so
        # the serial DMA pipe stays clear for rope tables/swaps early on.
        wo_sb = wp8.tile([128, NT, E], FP8, tag="W8", name="w_Wo")
        wxr_sb = wp8.tile([128, NT, E], FP8, tag="W8", name="w_Wxr")
        wyr_sb = wp8.tile([128, NT, E], FP8, tag="W8", name="w_Wyr")
        wxzh_sb = wp8.tile([128, NT, E], FP8, tag="W8", name="w_Wxzh")
        wxzh2_sb = wp8.tile([128, NT, E], FP8, tag="W8", name="w_Wxzh2")
        wxzr_sb = wp8.tile([128, NT, E], FP8, tag="W8", name="w_WxzR") if Z3 else None
        wyz_sb = wp8.tile([128, NT, E], FP8, tag="W8", name="w_Wyz")
        wxg_sb = wp8.tile([128, NT, E], FP8, tag="W8", name="w_Wxg")
        wyg_sb = wp8.tile([128, NT, E], FP8, tag="W8", name="w_Wyg")
        xl8 = res.tile([128, NT, S], FP8, tag="xl8")

        def emit_late_loads():
            # Emitted after the P3 loop: SP-queue FIFO order keeps these 9MB
            # behind the latency-critical rope swap DMAs on the serial pipe.
            pairs = [(wo_sb, "Wo"), (wxr_sb, "Wxr"), (wyr_sb, "Wyr"),
                     (xl8, None), (wxzh_sb, "Wxzh"), (wxzh2_sb, "Wxzh2")]
            if Z3:
                pairs.append((wxzr_sb, "WxzR"))
            pairs += [(wyz_sb, "Wyz"), (wxg_sb, "Wxg"), (wyg_sb, "Wyg")]
            for t, nm in pairs:
                if nm is None:
                    nc.sync.dma_start(out=t, in_=r3(xl8_d.ap()))
                else:
                    nc.sync.dma_start(
                        out=t, in_=w8_d[nm].ap().rearrange("p (t e) -> p t e", t=NT))

        # ===== P2: v = x @ Wv  (seq-major, fp8 out; dequant split Act/DVE) ==
        for st in range(NT):
            ss = slice(st * 128, (st + 1) * 128)
            ps = psum.tile([128, 1024], F32, tag="sc", bufs=2, name="ps_v")
            for c in range(2):
                sl = slice(c * 512, (c + 1) * 512)
                for kp in range(NP):
                    nc.tensor.matmul(
                        ps[:, sl], lhsT=xT8[:, 2 * kp:2 * kp + 2, ss],
                        rhs=wv_sb[:, 2 * kp:2 * kp + 2, sl],
                        start=(kp == 0), stop=(kp == NP - 1), perf_mode=DR)
            nc.scalar.activation(vsb8[:, st, :], ps, AF.Copy, scale=1.0 / WS)

        # ====== P3+P4 fused: q/k proj + RoPE, with per-head attention ======
        # Emitting scores/exp/av for head h as soon as tile h is rotated puts
        # the attention's DVE/Pool/Act ops early in those engines' in-order
        # queues; PE parking on not-yet-ready scores is absorbed by P3's PE
        # slack (rope is DVE/DMA-paced).
        p3_ctx = tc.tile_pool(name="p3", bufs=1)
        p3 = p3_ctx.__enter__()
        p4_ctx = tc.tile_pool(name="p4", bufs=1)
        p4 = p4_ctx.__enter__()
        ctx8 = res.tile([128, NT, S], FP8, tag="ctx8")  # 16*ctx in fp8
        EXPB = 3  # expT ring size; masked-block zeros written once/buffer

        def emit_scores(h):
            expT = p4.tile([128, NT, S], FP8, tag="expT", bufs=EXPB)
            for jt in range(NT):
                i0 = jt * 128
                ps = psum.tile([128, 1024], F32, tag="sc", bufs=2, name="ps_sc")
                if i0 < 512:
                    nc.tensor.matmul(
                        ps[:, i0:512],
                        lhsT=kr[:, h, i0:i0 + 128],
                        rhs=qr[:, h, i0:512], start=True, stop=True)
                    nc.tensor.matmul(
                        ps[:, 512:1024],
                        lhsT=kr[:, h, i0:i0 + 128],
                        rhs=qr[:, h, 512:1024], start=True, stop=True)
                else:
                    nc.tensor.matmul(
                        ps[:, i0:1024],
                        lhsT=kr[:, h, i0:i0 + 128],
                        rhs=qr[:, h, i0:1024], start=True, stop=True)
                nc.scalar.activation(expT[:, jt, i0:1024], ps[:, i0:1024],
                                     AF.Exp, scale=SCALE)
                nc.gpsimd.affine_select(
                    out=expT[:, jt, i0:i0 + 128], in_=expT[:, jt, i0:i0 + 128],
                    pattern=[[1, 128]], compare_op=ALU.is_ge,
                    fill=0.0, base=0, channel_multiplier=-1)
                if jt % 2 == 1 and h < EXPB:
                    nc.vector.memset(expT[:, jt, i0 - 128:i0], 0.0)
            return expT

        def emit_av(h, expT):
            for c in range(2):
                cs, ce = c * 512, (c + 1) * 512
                jps = [jp for jp in range(NP) if jp * 256 < ce]
                dps = psum.tile([128, 512], F32, tag="dc", bufs=2, name="ps_den")
                for n, jp in enumerate(jps):
                    a = max(jp * 256, cs)
                    nc.tensor.matmul(
                        dps[:, a - cs:512], lhsT=ones8,
                        rhs=expT[:, 2 * jp:2 * jp + 2, a:ce],
                        start=(n == 0), stop=(n == len(jps) - 1), perf_mode=DR)
                rf = p4.tile([128, 512], F32, tag="rf", bufs=1)
                nc.vector.reciprocal_approx_fast(out=rf, in_=dps)
                cps = psum.tile([128, 512], F32, tag="dc", bufs=2, name="ps_ctx")
                for n, jp in enumerate(jps):
                    a = max(jp * 256, cs)
                    nc.tensor.matmul(
                        cps[:, a - cs:512],
                        lhsT=vsb8[:, 2 * jp:2 * jp + 2, h * 128:(h + 1) * 128],
                        rhs=expT[:, 2 * jp:2 * jp + 2, a:ce],
                        start=(n == 0), stop=(n == len(jps) - 1), perf_mode=DR)
                nc.vector.tensor_mul(ctx8[:, h, cs:ce], cps, rf)

        if True:
            def proj_tile(t, w_sb, brow_off, on_act=True):
                # psum chains 512-wide; convert to bf16 qs; DMA pair-swap
                qs = p3.tile([128, S], BF16, tag="qs", bufs=3)
                for c in range(2):
                    sl = slice(c * 512, (c + 1) * 512)
                    ps = psum.tile([128, 512], F32, tag="a", bufs=2, name="ps_qk")
                    for kp in range(NP):
                        nc.tensor.matmul(
                            ps, lhsT=w_sb[:, 2 * kp:2 * kp + 2, t * 128:(t + 1) * 128],
                            rhs=xT8[:, 2 * kp:2 * kp + 2, sl],
                            start=(kp == 0), stop=(kp == NP - 1 and zb), perf_mode=DR)
                    if not zb:
                        nc.tensor.matmul(
                            ps, lhsT=bqk_row[:, brow_off + t * 128:brow_off + (t + 1) * 128],
                            rhs=ones1_b[:, sl], start=False, stop=True)
                    if c == (0 if on_act else 1):
                        nc.scalar.activation(qs[:, sl], ps, AF.Copy, scale=1.0 / WS)
                    else:
                        nc.vector.tensor_scalar(out=qs[:, sl], in0=ps,
                                                scalar1=1.0 / WS, scalar2=None,
                                                op0=ALU.mult)
                qss = p3.tile([128, S], BF16, tag="qss", bufs=3)
                qs_r = qs[:].rearrange("(h two) f -> h two f", two=2)
                qss_r = qss[:].rearrange("(h two) f -> h two f", two=2)
                nc.sync.dma_start(out=qss_r[:, 0, :], in_=qs_r[:, 1, :])
                nc.sync.dma_start(out=qss_r[:, 1, :], in_=qs_r[:, 0, :])
                return qs, qss

            def rotate(t, qs, qss, cos_t, sin_t, dst, add_on_pool=True):
                t1 = p3.tile([128, S], BF16, tag="t1", bufs=2)
                nc.vector.tensor_mul(t1, qs, cos_t[:, t, :])
                t2 = p3.tile([128, S], BF16, tag="t2", bufs=2)
                nc.vector.tensor_mul(t2, qss, sin_t[:, t, :])
                if add_on_pool:
                    nc.gpsimd.tensor_add(dst[:, t, :], t1, t2)
                else:
                    nc.vector.tensor_add(dst[:, t, :], t1, t2)

            pend_h = []

            def head_ready(r):
                pend_h.append((r, emit_scores(r)))
                if len(pend_h) > 2:
                    ph, pexp = pend_h.pop(0)
                    emit_av(ph, pexp)

            if share_qk:
                pend = []
                for t in range(NT):
                    q_t = proj_tile(t, wq_sb, 0, on_act=True)
                    k_t = proj_tile(t, wk_sb, E, on_act=False)
                    if 0 <= t < 6:
                        load_tab_chunk(t + 2, t + 3)
                    pend.append((t, q_t, k_t))
                    if len(pend) > 2:  # distance-2: swap DMA latency headroom
                        pt, pq, pk = pend.pop(0)
                        rotate(pt, pq[0], pq[1], cosq, sinq, qr, False)
                        rotate(pt, pk[0], pk[1], cosq, sinq, kr, False)
                        head_ready(pt)
                for pt, pq, pk in pend:
                    rotate(pt, pq[0], pq[1], cosq, sinq, qr, False)
                    rotate(pt, pk[0], pk[1], cosq, sinq, kr, False)
                    head_ready(pt)
                emit_late_loads()
                for ph, pexp in pend_h:
                    emit_av(ph, pexp)
            else:
                for t in range(NT):
                    q_t = proj_tile(t, wq_sb, 0)
                    if t == 0:
                        load_tab_chunk(2, 5)
                    if t == 2:
                        load_tab_chunk(5, NT)
                    rotate(t, q_t[0], q_t[1], cosq, sinq, qr, False)
                    if t == NT - 1:
                        emit_late_loads()
                cosk = tab.tile([128, NT, S], BF16, tag="cosq")
                nc.sync.dma_start(out=cosk, in_=r3(cosk_d.ap()))
                sink = tab.tile([128, NT, S], BF16, tag="sinq")
                nc.sync.dma_start(out=sink, in_=r3(sink_d.ap()))
                for t in range(NT):
                    k_t = proj_tile(t, wk_sb, E)
                    rotate(t, k_t[0], k_t[1], cosk, sink, kr)
                    head_ready(t)
                for ph, pexp in pend_h:
                    emit_av(ph, pexp)

        p4_ctx.__exit__(None, None, None)
        p3_ctx.__exit__(None, None, None)
        tab_ctx.__exit__(None, None, None)

        # =========== P5: y8 = 8*relu(ctx Wo + bo')  (feature-major) =========
        mid_ctx.__exit__(None, None, None)
        res2 = top.enter_context(tc.tile_pool(name="res2", bufs=1))
        yT8 = res2.tile([128, NT, S], FP8, tag="yT8")    # 8*y in fp8
        rx = res2.tile([128, NT, S], FP8, tag="rx")      # (r*x)^T fp8
        for t in range(NT):
            for c in range(2):
                sl = slice(c * 512, (c + 1) * 512)
                ps = psum.tile([128, 512], F32, tag=("a" if c == 0 else "dc"),
                               bufs=2, name="ps_y")
                for kp in range(NP):
                    nc.tensor.matmul(
                        ps, lhsT=wo_sb[:, 2 * kp:2 * kp + 2, t * 128:(t + 1) * 128],
                        rhs=ctx8[:, 2 * kp:2 * kp + 2, sl],
                        start=(kp == 0), stop=(kp == NP - 1), perf_mode=DR)
                if zb:
                    # bo' == 0: relu(s*ps) = s*max(ps, 0); alternate engines
                    # so the post-attention relu ladder runs two-wide
                    if c == 0:
                        nc.scalar.activation(yT8[:, t, sl], ps, AF.Relu,
                                             scale=YS / (WS * CS))
                    else:
                        nc.vector.tensor_scalar(out=yT8[:, t, sl], in0=ps,
                                                scalar1=0.0, scalar2=YS / (WS * CS),
                                                op0=ALU.max, op1=ALU.mult)
                else:
                    nc.scalar.activation(yT8[:, t, sl], ps, AF.Relu,
                                         bias=bo8_sb[:, t:t + 1],
                                         scale=YS / (WS * CS))

        # ===== P6: r = sigmoid(x Wxr + y Wyr); rx = r * xT (bf16) ============
        with tc.tile_pool(name="p6", bufs=1) as p6:
            for t in range(NT):
                rt = p6.tile([128, S], BF16, tag="rt", bufs=2)
                for c in range(2):
                    sl = slice(c * 512, (c + 1) * 512)
                    ps = psum.tile([128, 512], F32, tag=("a" if c == 0 else "dc"),
                                   bufs=2, name="ps_r")
                    for kp in range(NP):
                        nc.tensor.matmul(
                            ps, lhsT=wxr_sb[:, 2 * kp:2 * kp + 2, t * 128:(t + 1) * 128],
                            rhs=xT8[:, 2 * kp:2 * kp + 2, sl],
                            start=(kp == 0), stop=False, perf_mode=DR)
                    for kp in range(NP):
                        nc.tensor.matmul(
                            ps, lhsT=wyr_sb[:, 2 * kp:2 * kp + 2, t * 128:(t + 1) * 128],
                            rhs=yT8[:, 2 * kp:2 * kp + 2, sl],
                            start=False, stop=(kp == NP - 1), perf_mode=DR)
                    nc.scalar.activation(rt[:, sl], ps, AF.Sigmoid, scale=1.0 / WS)
                nc.vector.tensor_mul(rx[:, t, :], rt, xT8[:, t, :])

        # =========== P7: z/h + gated combine (seq-major, single pass) ========
        # xf loads + out stores ride the Act HWDGE queue to dodge SP-queue
        # head-of-line blocking behind the late weight loads.
        with tc.tile_pool(name="p7", bufs=1) as p7:
            xfs = []

            def load_xf(st):
                xf = p7.tile([128, E], F32, tag="xf", bufs=3)
                nc.gpsimd.dma_start(out=xf, in_=xb_d.ap()[st * 128:(st + 1) * 128, :])
                xfs.append(xf)

            load_xf(0)
            load_xf(1)
            for st in range(NT):
                ss = slice(st * 128, (st + 1) * 128)
                if st + 2 < NT:
                    load_xf(st + 2)
                xf = xfs[st]
                ot = p7.tile([128, E], F32, tag="ot", bufs=2)
                zps = psum.tile([128, 1024], F32, tag="sc", bufs=2, name="ps_z")
                for c in range(2):
                    sl = slice(c * 512, (c + 1) * 512)
                    chains = [(xT8, wxzh_sb), (xl8, wxzh2_sb)]
                    if Z3:
                        chains.append((xT8, wxzr_sb))
                    chains.append((yT8, wyz_sb))
                    for n, (lt, wt) in enumerate(chains):
                        for kp in range(NP):
                            nc.tensor.matmul(
                                zps[:, sl], lhsT=lt[:, 2 * kp:2 * kp + 2, ss],
                                rhs=wt[:, 2 * kp:2 * kp + 2, sl],
                                start=(n == 0 and kp == 0),
                                stop=(zb and n == len(chains) - 1 and kp == NP - 1),
                                perf_mode=DR)
                    if not zb:
                        nc.tensor.matmul(zps[:, sl], lhsT=ones1_b[:, 0:128],
                                         rhs=bxz_row[:, sl], start=False, stop=True)
                zt = p7.tile([128, E], F32, tag="zt", bufs=2)
                nc.scalar.activation(zt, zps, AF.Sigmoid, scale=1.0 / WS)
                hps = psum.tile([128, 1024], F32, tag="sc", bufs=2, name="ps_h")
                for c in range(2):
                    sl = slice(c * 512, (c + 1) * 512)
                    for kp in range(NP):
                        nc.tensor.matmul(hps[:, sl], lhsT=rx[:, 2 * kp:2 * kp + 2, ss],
                                         rhs=wxg_sb[:, 2 * kp:2 * kp + 2, sl],
                                         start=(kp == 0), stop=False, perf_mode=DR)
                    for kp in range(NP):
                        nc.tensor.matmul(hps[:, sl], lhsT=yT8[:, 2 * kp:2 * kp + 2, ss],
                                         rhs=wyg_sb[:, 2 * kp:2 * kp + 2, sl],
                                         start=False, stop=(kp == NP - 1), perf_mode=DR)
                # combine + store per 512-half: the tail (tanh -> diff ->
                # gate -> add -> store) pipelines across halves, cutting the
                # end-of-kernel serial chain roughly in half
                ht = p7.tile([128, E], F32, tag="ht", bufs=2)
                dt = p7.tile([128, E], F32, tag="dt", bufs=2)
                zd = p7.tile([128, E], F32, tag="zd", bufs=2)
                # last tile's tail chain runs at 256 granularity so the
                # final store starts as early as possible
                W = 512 if st < NT - 1 else 256
                for c in range(E // W):
                    sl = slice(c * W, (c + 1) * W)
                    nc.scalar.activation(ht[:, sl], hps[:, sl], AF.Tanh,
                                         scale=1.0 / WS)
                    if st < NT - 1:
                        nc.gpsimd.tensor_sub(dt[:, sl], ht[:, sl], xf[:, sl])
                    else:
                        nc.vector.tensor_sub(dt[:, sl], ht[:, sl], xf[:, sl])
                    nc.vector.tensor_mul(zd[:, sl], zt[:, sl], dt[:, sl])
                    nc.vector.tensor_add(ot[:, sl], xf[:, sl], zd[:, sl])
                    nc.sync.dma_start(out=out_d.ap()[ss, sl], in_=ot[:, sl])

    nc.compile()
    return nc


# ---------------- host-side packing -----------------------------------------

def _pack_w(w, scale, npdt):
    return np.ascontiguousarray(
        (np.asarray(w, np.float32) * scale).astype(npdt)
        .reshape(NT, 128, E).transpose(1, 0, 2).reshape(128, NT * E))


def _pack_fm(m, npdt):
    # [E, S]-logical feature-major -> [128, NT*S]
    return np.ascontiguousarray(
        m.astype(npdt).reshape(NT, 128, S).transpose(1, 0, 2).reshape(128, NT * S))


def _pack_bias_fm(b, scale=1.0):
    return np.ascontiguousarray(
        (np.asarray(b, np.float32) * scale).reshape(NT, 128).T)


_INV = None


def _inv_pair():
    global _INV
    if _INV is None:
        inv = 1.0 / (10000.0 ** (np.arange(0, E, 2, dtype=np.float32) / np.float32(E)))
        _INV = np.repeat(inv.astype(np.float64), 2)  # pair-expanded [E]
    return _INV


_SIGN = None


def _sign_pair():
    # sin sign pattern: -1 on even features, +1 on odd (shuffle drops the
    # pmat negation, so the sin table carries it)
    global _SIGN
    if _SIGN is None:
        s = np.ones(E, np.float64)
        s[0::2] = -1.0
        _SIGN = s
    return _SIGN


def _tables(idx):
    f = _inv_pair()[:, None] * idx.astype(np.float64)[None, :]  # [E, S]
    return (_pack_fm(np.cos(f).astype(np.float32), NPBF16),
            _pack_fm((np.sin(f) * _sign_pair()[:, None]).astype(np.float32), NPBF16))


def make_in_maps(inputs, share_qk, zb):
    x = np.asarray(inputs["x"], dtype=np.float32)
    qi = np.asarray(inputs["query_index"])
    ki = np.asarray(inputs["key_index"])
    bo_eff = (np.asarray(inputs["bo"], np.float32)
              + np.asarray(inputs["bv"], np.float32)
              @ np.asarray(inputs["Wo"], np.float32))
    common = {
        "ones8": np.full((128, 256), 1.0 / CS, NPFP8),
    }
    if not zb:
        common["bo8"] = _pack_bias_fm(bo_eff, YS)
        bqk = np.concatenate([np.asarray(inputs["bq"], np.float32),
                              np.asarray(inputs["bk"], np.float32)])
        common["bqk_row"] = (bqk * WS).astype(NPBF16).reshape(1, 2 * E)
        common["bxz_row"] = (np.asarray(inputs["bxz"], np.float32) * WS
                             ).astype(NPBF16).reshape(1, E)
    for nm in ("Wq", "Wk", "Wv", "Wo", "Wxr", "Wxg"):
        common[nm] = _pack_w(inputs[nm], WS, NPFP8)
    for nm in ("Wyr", "Wyz", "Wyg"):
        common[nm] = _pack_w(inputs[nm], WS / YS, NPFP8)
    wxz = np.asarray(inputs["Wxz"], np.float32)
    whi8 = (wxz * WS).astype(NPFP8)
    common["Wxzh"] = _pack_w(whi8.astype(np.float32), 1.0, NPFP8)
    common["Wxzh2"] = _pack_w(wxz, 2.0, NPFP8)
    if Z3:
        common["WxzR"] = _pack_w(wxz * WS - whi8.astype(np.float32), 1.0, NPFP8)
    in_maps = []
    for b in range(B):
        m = dict(common)
        xb = np.ascontiguousarray(x[b])
        m["xb"] = xb
        xt = xb.T  # [E, S]
        xh8 = xt.astype(NPFP8)
        m["xt8"] = _pack_fm(xh8.astype(np.float32), NPFP8)
        m["xl8"] = _pack_fm((xt - xh8.astype(np.float32)) * 16.0, NPFP8)
        m["cosq"], m["sinq"] = _tables(qi[b])
        if not share_qk:
            m["cosk"], m["sink"] = _tables(ki[b])
        in_maps.append(m)
    return in_maps


def kernel(**inputs):
    qi = np.asarray(inputs["query_index"])
    ki = np.asarray(inputs["key_index"])
    share_qk = bool(np.array_equal(qi, ki))
    zb = not (np.any(np.asarray(inputs["bq"])) or np.any(np.asarray(inputs["bk"]))
              or np.any(np.asarray(inputs["bxz"])) or np.any(np.asarray(inputs["bo"]))
              or np.any(np.asarray(inputs["bv"])))

    key = ("k", share_qk, zb)
    if key not in _COMPILED:
        _COMPILED[key] = _build(share_qk, zb)
    nc = _COMPILED[key]

    in_maps = make_in_maps(inputs, share_qk, zb)
    global _dbg_in_maps
    _dbg_in_maps = in_maps
    res = bass_utils.run_bass_kernel_spmd(nc, in_maps, core_ids=list(range(NC)))
    out = np.stack([res.results[b]["out"] for b in range(B)]).astype(np.float32)
    return out


# revision 86
# speedup vs baseline: 1.0096x; 1.0091x over previous
# Trainium2 Bass kernel for nn_EpisodeMultiheadAttentionBlock.
# B=8, S=1024, E=1024, H=8 heads, HD=128. Data-parallel over batch: core b
# computes batch element b. Self-contained: only needs /opt/trn_rl_repo on path.
#
# v5 (~144us cost-model, vs 197us baseline). Key design points:
#  - RoPE pair-shuffle via SBUF->SBUF DMA (sign folded into the sin table):
#    no pmat matmuls on PE, no psum->sbuf shuffle copies on Act.
#  - The cost model serializes ALL DMA transfers into one FIFO pipe, so
#    transfer ORDER is managed explicitly: minimal early loads (xt8/Wv/Wq/
#    cos+sin tiles 0-1/Wk), rope tables streamed per-tile inside the P3
#    loop, the 9MB of late weights emitted after the last rope swap, and
#    Wyg last on SP (its buffer frees only when attention ends).
#  - P2(v)/P3(qk+rope)/P4(attention) emission is FUSED: scores/exp/av for
#    head h are emitted as soon as tile h is rotated, so attention's
#    DVE/Act/Pool ops sit early in those in-order queues; PE parking on
#    not-yet-ready scores is absorbed by P3's PE slack.
#  - bv/bo folded exactly into bo' = bo + bv@Wo on host (softmax rows sum
#    to 1, so the v-bias passes through attention additively).
#  - bq/bk/bxz/bo handled by rank-1 matmuls / Act bias only when nonzero
#    (the common setup has all-zero biases -> "zb" fast variant).
#  - z-gate x-side: 2 fp8 DoubleRow chains (hi + activation-residual);
#    1 chain fails the 2e-2 gate, Z3 restores the 3-chain variant.
#  - PSUM (8 banks): proj "a" 2 + scores "sc" 4 + den/ctx "dc" 2 during
#    the stream; P5/P6 alternate chains across a/dc for a 4-deep ladder;
#    P7 z/h reuse "sc".
#  - Elementwise spread: Act gets q-convert halves, v-dequant and all
#    activations (exp is its 43us floor); DVE gets k-convert halves, rope
#    muls/adds, reciprocal+ctx-mul, relu halves, gate combines; Pool gets
#    causal affine_select and P7's (h-x). Masked-block zeros are written
#    once per expT buffer, not once per head.
#  - P7 combine+store runs per 512-half (256 on the last tile) to shorten
#    the end-of-kernel serial chain; xf loads ride the gpsimd SWDGE queue.
import sys
import numpy as np

sys.path.insert(0, "/opt/trn_rl_repo")

import ml_dtypes  # noqa: E402
import concourse.bass as bass  # noqa: E402
import concourse.mybir as mybir  # noqa: E402
import concourse.tile as tile  # noqa: E402
from concourse import bacc  # noqa: E402
from concourse import bass_utils  # noqa: E402

B, S, E, H = 8, 1024, 1024, 8
HD = E // H  # 128
NT = E // 128  # 8 e-tiles / s-tiles
NP = NT // 2  # 4 DoubleRow k-tile pairs
NC = 8  # cores
BF16 = mybir.dt.bfloat16
F32 = mybir.dt.float32
FP8 = mybir.dt.float8e4
AF = mybir.ActivationFunctionType
DR = mybir.MatmulPerfMode.DoubleRow
ALU = mybir.AluOpType
NPBF16 = ml_dtypes.bfloat16
NPFP8 = ml_dtypes.float8_e4m3

WS = 32.0  # weight pre-scale for fp8 weights
YS = 8.0   # y stored as 8*y in fp8
CS = 16.0  # ctx stored as 16*ctx in fp8
Z3 = False  # 3-chain z x-side (precision fallback)

_COMPILED = {}


def _build(share_qk: bool, zb: bool):
    nc = bacc.Bacc("TRN2", target_bir_lowering=False, debug=False, num_devices=NC)

    # ---- DRAM tensors -------------------------------------------------------
    xb_d = nc.dram_tensor("xb", [S, E], F32, kind="ExternalInput")
    xt8_d = nc.dram_tensor("xt8", [128, NT * S], FP8, kind="ExternalInput")
    xl8_d = nc.dram_tensor("xl8", [128, NT * S], FP8, kind="ExternalInput")
    wnames = ["Wq", "Wk", "Wv", "Wo", "Wxr", "Wyr", "Wyz", "Wxg", "Wyg",
              "Wxzh", "Wxzh2"]
    if Z3:
        wnames.append("WxzR")
    w8_d = {nm: nc.dram_tensor(nm, [128, NT * E], FP8, kind="ExternalInput")
            for nm in wnames}
    if not zb:
        bo8_d = nc.dram_tensor("bo8", [128, NT], F32, kind="ExternalInput")
    ones8_d = nc.dram_tensor("ones8", [128, 256], FP8, kind="ExternalInput")
    cosq_d = nc.dram_tensor("cosq", [128, NT * S], BF16, kind="ExternalInput")
    sinq_d = nc.dram_tensor("sinq", [128, NT * S], BF16, kind="ExternalInput")
    if not share_qk:
        cosk_d = nc.dram_tensor("cosk", [128, NT * S], BF16, kind="ExternalInput")
        sink_d = nc.dram_tensor("sink", [128, NT * S], BF16, kind="ExternalInput")
    if not zb:
        bqk_row_d = nc.dram_tensor("bqk_row", [1, 2 * E], BF16, kind="ExternalInput")
        bxz_row_d = nc.dram_tensor("bxz_row", [1, E], BF16, kind="ExternalInput")
    out_d = nc.dram_tensor("out", [S, E], F32, kind="ExternalOutput")

    SCALE = 1.0 / float(np.sqrt(HD))

    def r3(ap):
        return ap.rearrange("p (t s) -> p t s", t=NT)

    with tile.TileContext(nc) as tc:
      from contextlib import ExitStack

      with ExitStack() as top:
        res = top.enter_context(tc.tile_pool(name="res", bufs=1))
        consts = top.enter_context(tc.tile_pool(name="consts", bufs=1))
        wp8 = top.enter_context(tc.tile_pool(name="wp8", bufs=8))
        psum = top.enter_context(tc.tile_pool(name="psum", bufs=1, space="PSUM"))

        def load_w8(nm):
            t = wp8.tile([128, NT, E], FP8, tag="W8", name=f"w_{nm}")
            nc.sync.dma_start(out=t, in_=w8_d[nm].ap().rearrange("p (t e) -> p t e", t=NT))
            return t

        # ------- loads in prefetch order: what P2 needs first ---------------
        ones8 = consts.tile([128, 2, 128], FP8, tag="ones8")  # value 1/CS
        bo8_sb = None
        if not zb:
            bqk_row = consts.tile([1, 2 * E], BF16, tag="bqk_row")
            nc.sync.dma_start(out=bqk_row, in_=bqk_row_d.ap())
            bxz_row = consts.tile([1, E], BF16, tag="bxz_row")
            nc.sync.dma_start(out=bxz_row, in_=bxz_row_d.ap())
            ones1_b = consts.tile([1, 1024], BF16, tag="ones1_b")
            nc.vector.memset(ones1_b, 1.0)

        # PE p-state warmup: dummy matmuls fill the initial DMA-wait gap so
        # the first real chains run at full clock (ramp needs ~3us busy)
        wu = consts.tile([128, 512], BF16, tag="wu")
        nc.vector.memset(wu[:, 0:512], 0.0)
        for _ in range(14):
            wps = psum.tile([128, 512], F32, tag="a", bufs=2, name="ps_wu")
            nc.tensor.matmul(wps, lhsT=wu[:, 0:128], rhs=wu, start=True, stop=True)

        xT8 = res.tile([128, NT, S], FP8, tag="xT8")
        xt8_r = r3(xt8_d.ap())
        nc.sync.dma_start(out=xT8[:, 0:4, :], in_=xt8_r[:, 0:4, :])
        wv_sb = wp8.tile([128, NT, E], FP8, tag="W8", name="w_Wv")
        wv_r = w8_d["Wv"].ap().rearrange("p (t e) -> p t e", t=NT)
        nc.sync.dma_start(out=wv_sb[:, :, 0:512], in_=wv_r[:, :, 0:512])
        nc.sync.dma_start(out=xT8[:, 4:NT, :], in_=xt8_r[:, 4:NT, :])
        nc.sync.dma_start(out=wv_sb[:, :, 512:E], in_=wv_r[:, :, 512:E])
        wq_sb = load_w8("Wq")

        # mid tiles live through P4 only
        mid_ctx = tc.tile_pool(name="mid", bufs=1)
        mid = mid_ctx.__enter__()
        vsb8 = mid.tile([128, NT, E], FP8, tag="vsb8")   # v in fp8  [s, e]
        qr = mid.tile([128, NT, S], BF16, tag="qr")      # rope(q)^T
        kr = mid.tile([128, NT, S], BF16, tag="kr")      # rope(k)^T
        # rope tables live through P3 only; tiles 0-1 land before wk so the
        # first rotates (and thus head 0 of P4) start as early as possible
        tab_ctx = tc.tile_pool(name="tab", bufs=1)
        tab = tab_ctx.__enter__()
        cosq = tab.tile([128, NT, S], BF16, tag="cosq")
        sinq = tab.tile([128, NT, S], BF16, tag="sinq")
        cosq_r, sinq_r = r3(cosq_d.ap()), r3(sinq_d.ap())
        nc.sync.dma_start(out=cosq[:, 0:2, :], in_=cosq_r[:, 0:2, :])
        nc.sync.dma_start(out=sinq[:, 0:2, :], in_=sinq_r[:, 0:2, :])
        wk_sb = load_w8("Wk")
        # small consts off the latency-critical pipe front (first use ~35us)
        nc.sync.dma_start(out=ones8, in_=ones8_d.ap().rearrange("p (a b) -> p a b", a=2))
        if not zb:
            bo8_sb = consts.tile([128, NT], F32, tag="bo8")
            nc.sync.dma_start(out=bo8_sb, in_=bo8_d.ap())

        def load_tab_chunk(a, b):
            nc.sync.dma_start(out=cosq[:, a:b, :], in_=cosq_r[:, a:b, :])
            nc.sync.dma_start(out=sinq[:, a:b, :], in_=sinq_r[:, a:b, :])

        # Late tiles: allocated now, DMAs emitted mid-P3 behind a marker so
        # the serial DMA pipe stays clear for rope tables/swaps early on.
        wo_sb = wp8.tile([128, NT, E], FP8, tag="W8", name="w_Wo")
        wxr_sb = wp8.tile([128, NT, E], FP8, tag="W8", name="w_Wxr")
        wyr_sb = wp8.tile([128, NT, E], FP8, tag="W8", name="w_Wyr")
        wxzh_sb = wp8.tile([128, NT, E], FP8, tag="W8", name="w_Wxzh")
        wxzh2_sb = wp8.tile([128, NT, E], FP8, tag="W8", name="w_Wxzh2")
        wxzr_sb = wp8.tile([128, NT, E], FP8, tag="W8", name="w_WxzR") if Z3 else None
        wyz_sb = wp8.tile([128, NT, E], FP8, tag="W8", name="w_Wyz")
        wxg_sb = wp8.tile([128, NT, E], FP8, tag="W8", name="w_Wxg")
        wyg_sb = wp8.tile([128, NT, E], FP8, tag="W8", name="w_Wyg")
        xl8 = res.tile([128, NT, S], FP8, tag="xl8")

        def emit_late_loads():
            # Emitted after the P3 loop: SP-queue FIFO order keeps these 9MB
            # behind the latency-critical rope swap DMAs on the serial pipe.
            pairs = [(wo_sb, "Wo"), (wxr_sb, "Wxr"), (wyr_sb, "Wyr"),
                     (xl8, None), (wxzh_sb, "Wxzh"), (wxzh2_sb, "Wxzh2")]
            if Z3:
                pairs.append((wxzr_sb, "WxzR"))
            pairs += [(wyz_sb, "Wyz"), (wxg_sb, "Wxg"), (wyg_sb, "Wyg")]
            for t, nm in pairs:
                if nm is None:
                    nc.sync.dma_start(out=t, in_=r3(xl8_d.ap()))
                else:
                    nc.sync.dma_start(
                        out=t, in_=w8_d[nm].ap().rearrange("p (t e) -> p t e", t=NT))

        # ===== P2: v = x @ Wv  (seq-major, fp8 out; dequant split Act/DVE) ==
        for st in range(NT):
            ss = slice(st * 128, (st + 1) * 128)
            ps = psum.tile([128, 1024], F32, tag="sc", bufs=2, name="ps_v")
            for c in range(2):
                sl = slice(c * 512, (c + 1) * 512)
                for kp in range(NP):
                    nc.tensor.matmul(
                        ps[:, sl], lhsT=xT8[:, 2 * kp:2 * kp + 2, ss],
                        rhs=wv_sb[:, 2 * kp:2 * kp + 2, sl],
                        start=(kp == 0), stop=(kp == NP - 1), perf_mode=DR)
            nc.scalar.activation(vsb8[:, st, :], ps, AF.Copy, scale=1.0 / WS)

        # ====== P3+P4 fused: q/k proj + RoPE, with per-head attention ======
        # Emitting scores/exp/av for head h as soon as tile h is rotated puts
        # the attention's DVE/Pool/Act ops early in those engines' in-order
        # queues; PE parking on not-yet-ready scores is absorbed by P3's PE
        # slack (rope is DVE/DMA-paced).
        p3_ctx = tc.tile_pool(name="p3", bufs=1)
        p3 = p3_ctx.__enter__()
        p4_ctx = tc.tile_pool(name="p4", bufs=1)
        p4 = p4_ctx.__enter__()
        ctx8 = res.tile([128, NT, S], FP8, tag="ctx8")  # 16*ctx in fp8
        EXPB = 3  # expT ring size; masked-block zeros written once/buffer

        def emit_scores(h):
            expT = p4.tile([128, NT, S], FP8, tag="expT", bufs=EXPB)
            for jt in range(NT):
                i0 = jt * 128
                ps = psum.tile([128, 1024], F32, tag="sc", bufs=2, name="ps_sc")
                if i0 < 512:
                    nc.tensor.matmul(
                        ps[:, i0:512],
                        lhsT=kr[:, h, i0:i0 + 128],
                        rhs=qr[:, h, i0:512], start=True, stop=True)
                    nc.tensor.matmul(
                        ps[:, 512:1024],
                        lhsT=kr[:, h, i0:i0 + 128],
                        rhs=qr[:, h, 512:1024], start=True, stop=True)
                else:
                    nc.tensor.matmul(
                        ps[:, i0:1024],
                        lhsT=kr[:, h, i0:i0 + 128],
                        rhs=qr[:, h, i0:1024], start=True, stop=True)
                nc.scalar.activation(expT[:, jt, i0:1024], ps[:, i0:1024],
                                     AF.Exp, scale=SCALE)
                nc.gpsimd.affine_select(
                    out=expT[:, jt, i0:i0 + 128], in_=expT[:, jt, i0:i0 + 128],
                    pattern=[[1, 128]], compare_op=ALU.is_ge,
                    fill=0.0, base=0, channel_multiplier=-1)
                if jt % 2 == 1 and h < EXPB:
                    nc.vector.memset(expT[:, jt, i0 - 128:i0], 0.0)
            return expT

        def emit_av(h, expT):
            for c in range(2):
                cs, ce = c * 512, (c + 1) * 512
                jps = [jp for jp in range(NP) if jp * 256 < ce]
                dps = psum.tile([128, 512], F32, tag="dc", bufs=2, name="ps_den")
                for n, jp in enumerate(jps):
                    a = max(jp * 256, cs)
                    nc.tensor.matmul(
                        dps[:, a - cs:512], lhsT=ones8,
                        rhs=expT[:, 2 * jp:2 * jp + 2, a:ce],
                        start=(n == 0), stop=(n == len(jps) - 1), perf_mode=DR)
                rf = p4.tile([128, 512], F32, tag="rf", bufs=1)
                nc.vector.reciprocal_approx_fast(out=rf, in_=dps)
                cps = psum.tile([128, 512], F32, tag="dc", bufs=2, name="ps_ctx")
                for n, jp in enumerate(jps):
                    a = max(jp * 256, cs)
                    nc.tensor.matmul(
                        cps[:, a - cs:512],
                        lhsT=vsb8[:, 2 * jp:2 * jp + 2, h * 128:(h + 1) * 128],
                        rhs=expT[:, 2 * jp:2 * jp + 2, a:ce],
                        start=(n == 0), stop=(n == len(jps) - 1), perf_mode=DR)
                nc.vector.tensor_mul(ctx8[:, h, cs:ce], cps, rf)

        if True:
            def proj_tile(t, w_sb, brow_off, on_act=True):
                # psum chains 512-wide; convert to bf16 qs; DMA pair-swap
                qs = p3.tile([128, S], BF16, tag="qs", bufs=3)
                for c in range(2):
                    sl = slice(c * 512, (c + 1) * 512)
                    ps = psum.tile([128, 512], F32, tag="a", bufs=2, name="ps_qk")
                    for kp in range(NP):
                        nc.tensor.matmul(
                            ps, lhsT=w_sb[:, 2 * kp:2 * kp + 2, t * 128:(t + 1) * 128],
                            rhs=xT8[:, 2 * kp:2 * kp + 2, sl],
                            start=(kp == 0), stop=(kp == NP - 1 and zb), perf_mode=DR)
                    if not zb:
                        nc.tensor.matmul(
                            ps, lhsT=bqk_row[:, brow_off + t * 128:brow_off + (t + 1) * 128],
                            rhs=ones1_b[:, sl], start=False, stop=True)
                    if c == (0 if on_act else 1):
                        nc.scalar.activation(qs[:, sl], ps, AF.Copy, scale=1.0 / WS)
                    else:
                        nc.vector.tensor_scalar(out=qs[:, sl], in0=ps,
                                                scalar1=1.0 / WS, scalar2=None,
                                                op0=ALU.mult)
                qss = p3.tile([128, S], BF16, tag="qss", bufs=3)
                qs_r = qs[:].rearrange("(h two) f -> h two f", two=2)
                qss_r = qss[:].rearrange("(h two) f -> h two f", two=2)
                nc.sync.dma_start(out=qss_r[:, 0, :], in_=qs_r[:, 1, :])
                nc.sync.dma_start(out=qss_r[:, 1, :], in_=qs_r[:, 0, :])
                return qs, qss

            def rotate(t, qs, qss, cos_t, sin_t, dst, add_on_pool=True):
                t1 = p3.tile([128, S], BF16, tag="t1", bufs=2)
                nc.vector.tensor_mul(t1, qs, cos_t[:, t, :])
                t2 = p3.tile([128, S], BF16, tag="t2", bufs=2)
                nc.vector.tensor_mul(t2, qss, sin_t[:, t, :])
                if add_on_pool:
                    nc.gpsimd.tensor_add(dst[:, t, :], t1, t2)
                else:
                    nc.vector.tensor_add(dst[:, t, :], t1, t2)

            pend_h = []

            def head_ready(r):
                pend_h.append((r, emit_scores(r)))
                if len(pend_h) > 2:
                    ph, pexp = pend_h.pop(0)
                    emit_av(ph, pexp)

            if share_qk:
                pend = []
                for t in range(NT):
                    q_t = proj_tile(t, wq_sb, 0, on_act=True)
                    k_t = proj_tile(t, wk_sb, E, on_act=False)
                    if 0 <= t < 6:
                        load_tab_chunk(t + 2, t + 3)
                    pend.append((t, q_t, k_t))
                    if len(pend) > 2:  # distance-2: swap DMA latency headroom
                        pt, pq, pk = pend.pop(0)
                        rotate(pt, pq[0], pq[1], cosq, sinq, qr, False)
                        rotate(pt, pk[0], pk[1], cosq, sinq, kr, False)
                        head_ready(pt)
                for pt, pq, pk in pend:
                    rotate(pt, pq[0], pq[1], cosq, sinq, qr, False)
                    rotate(pt, pk[0], pk[1], cosq, sinq, kr, False)
                    head_ready(pt)
                emit_late_loads()
                for ph, pexp in pend_h:
                    emit_av(ph, pexp)
            else:
                for t in range(NT):
                    q_t = proj_tile(t, wq_sb, 0)
                    if t == 0:
                        load_tab_chunk(2, 5)
                    if t == 2:
                        load_tab_chunk(5, NT)
                    rotate(t, q_t[0], q_t[1], cosq, sinq, qr, False)
                    if t == NT - 1:
                        emit_late_loads()
                cosk = tab.tile([128, NT, S], BF16, tag="cosq")
                nc.sync.dma_start(out=cosk, in_=r3(cosk_d.ap()))
                sink = tab.tile([128, NT, S], BF16, tag="sinq")
                nc.sync.dma_start(out=sink, in_=r3(sink_d.ap()))
                for t in range(NT):
                    k_t = proj_tile(t, wk_sb, E)
                    rotate(t, k_t[0], k_t[1], cosk, sink, kr)
                    head_ready(t)
                for ph, pexp in pend_h:
                    emit_av(ph, pexp)

        p4_ctx.__exit__(None, None, None)
        p3_ctx.__exit__(None, None, None)
        tab_ctx.__exit__(None, None, None)

        # =========== P5: y8 = 8*relu(ctx Wo + bo')  (feature-major) =========
        mid_ctx.__exit__(None, None, None)
        res2 = top.enter_context(tc.tile_pool(name="res2", bufs=1))
        yT8 = res2.tile([128, NT, S], FP8, tag="yT8")    # 8*y in fp8
        rx = res2.tile([128, NT, S], FP8, tag="rx")      # (r*x)^T fp8
        for c in range(2):
            for t in range(NT):
                sl = slice(c * 512, (c + 1) * 512)
                ps = psum.tile([128, 512], F32, tag=("a" if c == 0 else "dc"),
                               bufs=2, name="ps_y")
                for kp in range(NP):
                    nc.tensor.matmul(
                        ps, lhsT=wo_sb[:, 2 * kp:2 * kp + 2, t * 128:(t + 1) * 128],
                        rhs=ctx8[:, 2 * kp:2 * kp + 2, sl],
                        start=(kp == 0), stop=(kp == NP - 1), perf_mode=DR)
                if zb:
                    # bo' == 0: relu(s*ps) = s*max(ps, 0); alternate engines
                    # so the post-attention relu ladder runs two-wide
                    if c == 0:
                        nc.scalar.activation(yT8[:, t, sl], ps, AF.Relu,
                                             scale=YS / (WS * CS))
                    else:
                        nc.vector.tensor_scalar(out=yT8[:, t, sl], in0=ps,
                                                scalar1=0.0, scalar2=YS / (WS * CS),
                                                op0=ALU.max, op1=ALU.mult)
                else:
                    nc.scalar.activation(yT8[:, t, sl], ps, AF.Relu,
                                         bias=bo8_sb[:, t:t + 1],
                                         scale=YS / (WS * CS))

        # ===== P6: r = sigmoid(x Wxr + y Wyr); rx = r * xT (bf16) ============
        with tc.tile_pool(name="p6", bufs=1) as p6:
            for t in range(NT):
                rt = p6.tile([128, S], BF16, tag="rt", bufs=2)
                for c in range(2):
                    sl = slice(c * 512, (c + 1) * 512)
                    ps = psum.tile([128, 512], F32, tag=("a" if c == 0 else "dc"),
                                   bufs=2, name="ps_r")
                    for kp in range(NP):
                        nc.tensor.matmul(
                            ps, lhsT=wxr_sb[:, 2 * kp:2 * kp + 2, t * 128:(t + 1) * 128],
                            rhs=xT8[:, 2 * kp:2 * kp + 2, sl],
                            start=(kp == 0), stop=False, perf_mode=DR)
                    for kp in range(NP):
                        nc.tensor.matmul(
                            ps, lhsT=wyr_sb[:, 2 * kp:2 * kp + 2, t * 128:(t + 1) * 128],
                            rhs=yT8[:, 2 * kp:2 * kp + 2, sl],
                            start=False, stop=(kp == NP - 1), perf_mode=DR)
                    nc.scalar.activation(rt[:, sl], ps, AF.Sigmoid, scale=1.0 / WS)
                nc.vector.tensor_mul(rx[:, t, :], rt, xT8[:, t, :])

        # =========== P7: z/h + gated combine (seq-major, single pass) ========
        # xf loads + out stores ride the Act HWDGE queue to dodge SP-queue
        # head-of-line blocking behind the late weight loads.
        with tc.tile_pool(name="p7", bufs=1) as p7:
            xfs = []

            def load_xf(st):
                xf = p7.tile([128, E], F32, tag="xf", bufs=3)
                nc.gpsimd.dma_start(out=xf, in_=xb_d.ap()[st * 128:(st + 1) * 128, :])
                xfs.append(xf)

            load_xf(0)
            load_xf(1)
            for st in range(NT):
                ss = slice(st * 128, (st + 1) * 128)
                if st + 2 < NT:
                    load_xf(st + 2)
                xf = xfs[st]
                ot = p7.tile([128, E], F32, tag="ot", bufs=2)
                zps = psum.tile([128, 1024], F32, tag="sc", bufs=2, name="ps_z")
                for c in range(2):
                    sl = slice(c * 512, (c + 1) * 512)
                    chains = [(xT8, wxzh_sb), (xl8, wxzh2_sb)]
                    if Z3:
                        chains.append((xT8, wxzr_sb))
                    chains.append((yT8, wyz_sb))
                    for n, (lt, wt) in enumerate(chains):
                        for kp in range(NP):
                            nc.tensor.matmul(
                                zps[:, sl], lhsT=lt[:, 2 * kp:2 * kp + 2, ss],
                                rhs=wt[:, 2 * kp:2 * kp + 2, sl],
                                start=(n == 0 and kp == 0),
                                stop=(zb and n == len(chains) - 1 and kp == NP - 1),
                                perf_mode=DR)
                    if not zb:
                        nc.tensor.matmul(zps[:, sl], lhsT=ones1_b[:, 0:128],
                                         rhs=bxz_row[:, sl], start=False, stop=True)
                zt = p7.tile([128, E], F32, tag="zt", bufs=2)
                nc.scalar.activation(zt, zps, AF.Sigmoid, scale=1.0 / WS)
                # h rides the 512-wide a/dc psums (free after P6) so the
                # z-stream (on sc) and h-stream run as parallel PE pipelines
                ht_pre = p7.tile([128, E], F32, tag="ht", bufs=2)
                for c in range(2):
                    sl = slice(c * 512, (c + 1) * 512)
                    hp = psum.tile([128, 512], F32, tag=("a" if c == 0 else "dc"),
                                   bufs=2, name="ps_h")
                    for kp in range(NP):
                        nc.tensor.matmul(hp, lhsT=rx[:, 2 * kp:2 * kp + 2, ss],
                                         rhs=wxg_sb[:, 2 * kp:2 * kp + 2, sl],
                                         start=(kp == 0), stop=False, perf_mode=DR)
                    for kp in range(NP):
                        nc.tensor.matmul(hp, lhsT=yT8[:, 2 * kp:2 * kp + 2, ss],
                                         rhs=wyg_sb[:, 2 * kp:2 * kp + 2, sl],
                                         start=False, stop=(kp == NP - 1), perf_mode=DR)
                    nc.scalar.activation(ht_pre[:, sl], hp, AF.Tanh, scale=1.0 / WS)
                # combine + store per 512-half: the tail (tanh -> diff ->
                # gate -> add -> store) pipelines across halves, cutting the
                # end-of-kernel serial chain roughly in half
                dt = p7.tile([128, E], F32, tag="dt", bufs=2)
                zd = p7.tile([128, E], F32, tag="zd", bufs=2)
                # last tile's tail chain runs at 256 granularity so the
                # final store starts as early as possible
                W = 512 if st < NT - 1 else 256
                for c in range(E // W):
                    sl = slice(c * W, (c + 1) * W)
                    if st < NT - 1:
                        nc.gpsimd.tensor_sub(dt[:, sl], ht_pre[:, sl], xf[:, sl])
                    else:
                        nc.vector.tensor_sub(dt[:, sl], ht_pre[:, sl], xf[:, sl])
                    nc.vector.tensor_mul(zd[:, sl], zt[:, sl], dt[:, sl])
                    nc.vector.tensor_add(ot[:, sl], xf[:, sl], zd[:, sl])
                    nc.sync.dma_start(out=out_d.ap()[ss, sl], in_=ot[:, sl])

    nc.compile()
    return nc


# ---------------- host-side packing -----------------------------------------

def _pack_w(w, scale, npdt):
    return np.ascontiguousarray(
        (np.asarray(w, np.float32) * scale).astype(npdt)
        .reshape(NT, 128, E).transpose(1, 0, 2).reshape(128, NT * E))


def _pack_fm(m, npdt):
    # [E, S]-logical feature-major -> [128, NT*S]
    return np.ascontiguousarray(
        m.astype(npdt).reshape(NT, 128, S).transpose(1, 0, 2).reshape(128, NT * S))


def _pack_bias_fm(b, scale=1.0):
    return np.ascontiguousarray(
        (np.asarray(b, np.float32) * scale).reshape(NT, 128).T)


_INV = None


def _inv_pair():
    global _INV
    if _INV is None:
        inv = 1.0 / (10000.0 ** (np.arange(0, E, 2, dtype=np.float32) / np.float32(E)))
        _INV = np.repeat(inv.astype(np.float64), 2)  # pair-expanded [E]
    return _INV


_SIGN = None


def _sign_pair():
    # sin sign pattern: -1 on even features, +1 on odd (shuffle drops the
    # pmat negation, so the sin table carries it)
    global _SIGN
    if _SIGN is None:
        s = np.ones(E, np.float64)
        s[0::2] = -1.0
        _SIGN = s
    return _SIGN


def _tables(idx):
    f = _inv_pair()[:, None] * idx.astype(np.float64)[None, :]  # [E, S]
    return (_pack_fm(np.cos(f).astype(np.float32), NPBF16),
            _pack_fm((np.sin(f) * _sign_pair()[:, None]).astype(np.float32), NPBF16))


def make_in_maps(inputs, share_qk, zb):
    x = np.asarray(inputs["x"], dtype=np.float32)
    qi = np.asarray(inputs["query_index"])
    ki = np.asarray(inputs["key_index"])
    bo_eff = (np.asarray(inputs["bo"], np.float32)
              + np.asarray(inputs["bv"], np.float32)
              @ np.asarray(inputs["Wo"], np.float32))
    common = {
        "ones8": np.full((128, 256), 1.0 / CS, NPFP8),
    }
    if not zb:
        common["bo8"] = _pack_bias_fm(bo_eff, YS)
        bqk = np.concatenate([np.asarray(inputs["bq"], np.float32),
                              np.asarray(inputs["bk"], np.float32)])
        common["bqk_row"] = (bqk * WS).astype(NPBF16).reshape(1, 2 * E)
        common["bxz_row"] = (np.asarray(inputs["bxz"], np.float32) * WS
                             ).astype(NPBF16).reshape(1, E)
    for nm in ("Wq", "Wk", "Wv", "Wo", "Wxr", "Wxg"):
        common[nm] = _pack_w(inputs[nm], WS, NPFP8)
    for nm in ("Wyr", "Wyz", "Wyg"):
        common[nm] = _pack_w(inputs[nm], WS / YS, NPFP8)
    wxz = np.asarray(inputs["Wxz"], np.float32)
    whi8 = (wxz * WS).astype(NPFP8)
    common["Wxzh"] = _pack_w(whi8.astype(np.float32), 1.0, NPFP8)
    common["Wxzh2"] = _pack_w(wxz, 2.0, NPFP8)
    if Z3:
        common["WxzR"] = _pack_w(wxz * WS - whi8.astype(np.float32), 1.0, NPFP8)
    in_maps = []
    for b in range(B):
        m = dict(common)
        xb = np.ascontiguousarray(x[b])
        m["xb"] = xb
        xt = xb.T  # [E, S]
        xh8 = xt.astype(NPFP8)
        m["xt8"] = _pack_fm(xh8.astype(np.float32), NPFP8)
        m["xl8"] = _pack_fm((xt - xh8.astype(np.float32)) * 16.0, NPFP8)
        m["cosq"], m["sinq"] = _tables(qi[b])
        if not share_qk:
            m["cosk"], m["sink"] = _tables(ki[b])
        in_maps.append(m)
    return in_maps


def kernel(**inputs):
    qi = np.asarray(inputs["query_index"])
    ki = np.asarray(inputs["key_index"])
    share_qk = bool(np.array_equal(qi, ki))
    zb = not (np.any(np.asarray(inputs["bq"])) or np.any(np.asarray(inputs["bk"]))
              or np.any(np.asarray(inputs["bxz"])) or np.any(np.asarray(inputs["bo"]))
              or np.any(np.asarray(inputs["bv"])))

    key = ("k", share_qk, zb)
    if key not in _COMPILED:
        _COMPILED[key] = _build(share_qk, zb)
    nc = _COMPILED[key]

    in_maps = make_in_maps(inputs, share_qk, zb)
    global _dbg_in_maps
    _dbg_in_maps = in_maps
    res = bass_utils.run_bass_kernel_spmd(nc, in_maps, core_ids=list(range(NC)))
    out = np.stack([res.results[b]["out"] for b in range(B)]).astype(np.float32)
    return out
